# revision 1
# baseline (speedup 1.0000x reference)
"""Trainium2 Bass kernel for nn_Affinity_Propagate — fp16 iteration state.

Algorithm (per batch image, one image per NeuronCore, 8 cores data-parallel):
    gate_wb[c] = shift_c(guidance[c])           (SPN shift, zero pad)
    w[c]       = gate_wb[c] / max(sum_c |gate_wb[c]|, eps)
    base       = (1 - sum_c w[c]) * blur
    r          = blur;  repeat prop_time times:  r = sum_c w[c]*shift_c(r) + base

Same structure as the fp32 baseline (rows on 120 SBUF partitions, 4 per
partition with 1-row/col zero halos; DVE products, PE identity-matmul
accumulation into PSUM fp32, ACT PSUM->SBUF copy-out, halo rows refreshed by
two partition-shifted SBUF DMAs per iteration), but the whole iteration
state (gates, r, products, base) is float16, which roughly doubles DVE
tensor_tensor throughput (2x_1P packed mode) — the baseline's bottleneck.

fp16 packing needs every DVE operand 4B-aligned with step-1 innermost runs,
so products cannot read r at odd column offsets (the +-1 column shifts of
the stencil). Instead each gate tile bakes its channel's column shift: gate
tiles are row-padded [P, RP, ROWW] with w_c placed at column offset 1+dx,
products multiply the FULL padded rows (flat [P, RP*ROWW], r read at offset
(1+dy)*ROWW -- always even), and the PE's rhs access pattern reads the
product plane at column offset 1+dx so values land at the right output
position. Out-of-range taps are zero because the r tile's halo columns stay
zero and gate borders are zeroed at load. Guidance/blur are uploaded as
fp16 (halves the HBM load traffic); the output is downloaded as fp16 and
cast to fp32 on host.

Measured (axon trn2, R-slope): 24-iteration loop ~118-120 us (~4.9 us/iter,
vs ~420 us for the fp32 baseline); absmax relative error ~9.7e-4 vs the
fp32 reference (fp16 gate/product/state rounding; tolerance is 2e-2).

build_nc knobs: repeat=N repeats the iteration loop in-NEFF (timing slopes);
whole_repeat=N repeats the entire body incl. loads+precompute (whole-kernel
slope); fuse=True merges each dy-group's products into one DVE op via a
step-0 operand dim (measured slower on HW than 8 separate ops -- the packed
mode appears not to engage for those APs -- so default False).
"""

import numpy as np
from contextlib import ExitStack

import concourse.bacc as bacc
import concourse.tile as tile
import concourse.mybir as mybir
from concourse.bass_utils import run_bass_kernel_spmd

H, W = 480, 640
B = 8
NCORES = 8
RP = 4                  # image rows per partition
P = H // RP             # 120 partitions used
NROW = RP + 2           # row slots incl. top/bottom halo
ROWW = W + 2            # row width incl. left/right halo (even)
FLAT = RP * W           # 2560 tight free elems per partition
FLATP = RP * ROWW       # 2568 padded free elems per partition
OFFSETS = [(-1, -1), (-1, 0), (-1, 1), (0, -1), (0, 1), (1, -1), (1, 0), (1, 1)]
EPS = 1e-4

F16 = mybir.dt.float16
F32 = mybir.dt.float32
ALU = mybir.AluOpType
ACTF = mybir.ActivationFunctionType

# product emission order: dy==0 first (no row-halo dep), then dy==+1 (down
# halo, written first), then dy==-1 (up halo, written last)
ORDERED_C = [3, 4, 5, 6, 7, 0, 1, 2]
GROUPS = [(0, [3, 4]), (1, [5, 6, 7]), (-1, [0, 1, 2])]   # (dy, channels)

MMCH, BANK, NMM = 320, 512, 8   # PSUM: 8 banks, 320-elem chunk per bank


def _rep_dim(ap, count):
    """Prefix a [step=0, count] free dim so one DVE op re-reads the same
    operand for each channel of a group."""
    ap = ap.unsqueeze(1).copy()
    ap.ap[1] = [0, count]
    return ap


def _load_shifted(nc, g_view, src2d, dy, dx):
    """DMA guidance channel (as [H, W] dram AP) shifted by (dy, dx) into the
    tight gate tile view g_view ([P, RP, W]); border elements are left
    untouched (pre-zeroed)."""
    r0 = max(0, -dy)            # dest flat row range [r0, r1)
    r1 = H - max(0, dy)
    x0 = max(0, -dx)            # dest col range [x0, x1)
    x1 = W - max(0, dx)
    p_start = (r0 + RP - 1) // RP
    p_end = r1 // RP
    if p_end > p_start:
        src = src2d[RP * p_start + dy:RP * p_end + dy, x0 + dx:x1 + dx]
        src = src.rearrange("(p j) w -> p j w", j=RP)
        nc.sync.dma_start(out=g_view[p_start:p_end, :, x0:x1], in_=src)
    if r0 % RP != 0:
        p = p_start - 1
        j0 = r0 - RP * p
        src = src2d[RP * p + j0 + dy:RP * (p + 1) + dy, x0 + dx:x1 + dx]
        src = src.rearrange("(p j) w -> p j w", j=RP - j0)
        nc.sync.dma_start(out=g_view[p:p + 1, j0:RP, x0:x1], in_=src)
    if r1 % RP != 0:
        p = p_end
        j1 = r1 - RP * p
        src = src2d[RP * p + dy:RP * p + j1 + dy, x0 + dx:x1 + dx]
        src = src.rearrange("(p j) w -> p j w", j=j1)
        nc.sync.dma_start(out=g_view[p:p + 1, 0:j1, x0:x1], in_=src)


def _emit(ctx, tc, guid, blur, ident_d, out_d, prop_time, repeat=1,
          split_first=2, fuse=False, merge_dx0=False, split_last=False,
          prod_bufs=6):
    nc = tc.nc

    const_pool = ctx.enter_context(tc.tile_pool(name="const", bufs=1))
    r_pool = ctx.enter_context(tc.tile_pool(name="rbuf", bufs=1))

    ident = const_pool.tile([P, P], F16, tag="ident", name="ident_sb")
    nc.sync.dma_start(out=ident[:], in_=ident_d)

    # baked gate tiles: w_c at column offset 1+dx within padded rows.
    # When fused, the channels of each dy-group live in one contiguous tile
    # so the group's products are a single DVE op.
    if fuse:
        grp_tiles = [
            const_pool.tile([P, len(ch) * FLATP], F16, tag=f"gg{gi}",
                            name=f"gg{gi}_sb")
            for gi, (dy, ch) in enumerate(GROUPS)
        ]
        gates = [None] * 8
        for gt, (dy, chans) in zip(grp_tiles, GROUPS):
            for ci, c in enumerate(chans):
                gates[c] = gt[:, ci * FLATP:(ci + 1) * FLATP]
        gate_mem = grp_tiles
    else:
        gtiles = [const_pool.tile([P, FLATP], F16, tag=f"g{c}", name=f"g{c}_sb")
                  for c in range(8)]
        gates = [t[:] for t in gtiles]
        gate_mem = gtiles
    base = const_pool.tile([P, FLAT], F16, tag="base", name="base_sb")

    rbufs = [r_pool.tile([P, NROW * ROWW], F16, tag=f"r{i}", name=f"r{i}_sb")
             for i in range(2)]
    for rb in rbufs:
        nc.gpsimd.memset(rb[:], 0.0)
    for g in gate_mem:
        nc.gpsimd.memset(g[:], 0.0)

    rviews = [rb[:].rearrange("p (r w) -> p r w", r=NROW) for rb in rbufs]

    # blur -> r0 center, then initial row-halo exchange
    blur_t = blur.rearrange("(p j) w -> p j w", j=RP)
    rv0 = rviews[0]
    nc.sync.dma_start(out=rv0[:, 1:1 + RP, 1:1 + W], in_=blur_t)
    nc.sync.dma_start(out=rv0[1:P, 0:1, :], in_=rv0[0:P - 1, RP:RP + 1, :])
    nc.sync.dma_start(out=rv0[0:P - 1, RP + 1:RP + 2, :], in_=rv0[1:P, 1:2, :])

    # ---- one-time precompute -------------------------------------------
    with tc.tile_pool(name="pretmp", bufs=1) as tmp_pool, \
         tc.tile_pool(name="prepsum", bufs=1, space="PSUM") as ppsum_pool:
        # tight SPN-shifted guidance loads
        gts = []
        for c, (dy, dx) in enumerate(OFFSETS):
            gt = tmp_pool.tile([P, FLAT], F16, tag=f"gt{c}", name=f"gt{c}_sb")
            gv = gt[:].rearrange("p (j w) -> p j w", j=RP)
            if dy == -1:
                nc.vector.memset(gv[:, 0:1, :], 0.0)
            elif dy == 1:
                nc.vector.memset(gv[:, RP - 1:RP, :], 0.0)
            if dx == -1:
                nc.vector.memset(gv[:, :, 0:1], 0.0)
            elif dx == 1:
                nc.vector.memset(gv[:, :, W - 1:W], 0.0)
            _load_shifted(nc, gv, guid[c], dy, dx)
            gts.append(gt)

        # S = sum_c g_c on the (idle) PE via identity matmuls
        spsum = ppsum_pool.tile([P, NMM * BANK], F32, tag="spsum", name="spsum_t")
        for ci, gt in enumerate(gts):
            for q in range(NMM):
                nc.tensor.matmul(spsum[:, q * BANK:q * BANK + MMCH], ident[:],
                                 gt[:, q * MMCH:(q + 1) * MMCH],
                                 start=(ci == 0), stop=(ci == 7))
        S = tmp_pool.tile([P, FLAT], F16, tag="rawsum", name="rawsum_sb")
        nc.scalar.activation(
            S[:].rearrange("p (q b) -> p q b", q=NMM),
            spsum[:].rearrange("p (q b) -> p q b", q=NMM)[:, :, 0:MMCH],
            ACTF.Copy)

        # A = sum_c |g_c| (abs on ScalarE, adds on DVE)
        A = tmp_pool.tile([P, FLAT], F16, tag="absum", name="absum_sb")
        nc.scalar.activation(A[:], gts[0][:], ACTF.Abs)
        for c in range(1, 8):
            abc = tmp_pool.tile([P, FLAT], F16, tag="abst", name="abst_sb", bufs=2)
            nc.scalar.activation(abc[:], gts[c][:], ACTF.Abs)
            nc.vector.tensor_tensor(A[:], A[:], abc[:], op=ALU.add)
        nc.vector.tensor_scalar_max(A[:], A[:], EPS)
        nc.vector.reciprocal(A[:], A[:])

        # baked gates: w'_c[:, :, 1+dx : 1+dx+W] = g_c * (1/A)
        Av = A[:].rearrange("p (j w) -> p j w", j=RP)
        for c, (dy, dx) in enumerate(OFFSETS):
            gw = gates[c].rearrange("p (j w) -> p j w", j=RP)
            nc.vector.tensor_tensor(
                gw[:, :, 1 + dx:1 + dx + W],
                gts[c][:].rearrange("p (j w) -> p j w", j=RP),
                Av, op=ALU.mult)

        # base = (1 - S/A) * blur
        nc.vector.tensor_tensor(S[:], S[:], A[:], op=ALU.mult)
        nc.vector.tensor_scalar(S[:], S[:], -1.0, 1.0, op0=ALU.mult, op1=ALU.add)
        d_center = rviews[0][:, 1:1 + RP, 1:1 + W]
        bview = base[:].rearrange("p (j w) -> p j w", j=RP)
        nc.vector.tensor_tensor(bview, S[:].rearrange("p (j w) -> p j w", j=RP),
                                d_center, op=ALU.mult)

    # ---- iteration loop -------------------------------------------------
    prod_pool = ctx.enter_context(
        tc.tile_pool(name="prod", bufs=5 if fuse else prod_bufs))
    psum_pool = ctx.enter_context(tc.tile_pool(name="acc", bufs=1, space="PSUM"))

    def mm_plane(psum, plane, dx, start, stop):
        """Accumulate a padded product plane into psum, reading the plane at
        column offset 1+dx so values land at the right output position.
        plane is [P, FLATP]; chunk q covers row j=q//2, cols [320*(q%2), +320)."""
        for q in range(NMM):
            j, h = q // 2, q % 2
            off = j * ROWW + 1 + dx + h * MMCH
            nc.tensor.matmul(psum[:, q * BANK:q * BANK + MMCH], ident[:],
                             plane[:, off:off + MMCH],
                             start=start, stop=stop)

    def mm_tight(psum, plane, start, stop):
        for q in range(NMM):
            nc.tensor.matmul(psum[:, q * BANK:q * BANK + MMCH], ident[:],
                             plane[:, q * MMCH:(q + 1) * MMCH],
                             start=start, stop=stop)

    out_t = out_d.rearrange("(p j) w -> p j w", j=RP)
    niter = prop_time * repeat
    cur, nxt = 0, 1
    for it in range(niter):
        final = it == niter - 1
        rb = rbufs[cur]
        psum = psum_pool.tile([P, NMM * BANK], F32, tag="psum", name="psum_t")

        mm_tight(psum, base, True, False)       # base opens each bank group

        if fuse:
            # one DVE op per dy-group; the r operand gets a step-0 outer dim
            for gi, (dy, chans) in enumerate(GROUPS):
                ncg = len(chans)
                pr = prod_pool.tile([P, 3 * FLATP], F16, tag="prod",
                                    name="prod_t")
                pv = pr[:, :ncg * FLATP].rearrange("p (c f) -> p c f", c=ncg)
                gv = gate_mem[gi][:].rearrange("p (c f) -> p c f", c=ncg)
                if gi == 0 and split_first:
                    # row-split the dy==0 group so DVE has work the moment
                    # each ACT row-copy of the previous iteration lands
                    for j in range(RP):
                        sl = slice(j * ROWW, (j + 1) * ROWW)
                        rsl = rb[:, (1 + j) * ROWW:(2 + j) * ROWW]
                        nc.vector.tensor_tensor(pv[:, :, sl], gv[:, :, sl],
                                                _rep_dim(rsl, ncg), op=ALU.mult)
                else:
                    rsl = rb[:, (1 + dy) * ROWW:(1 + dy) * ROWW + FLATP]
                    nc.vector.tensor_tensor(pv, gv, _rep_dim(rsl, ncg),
                                            op=ALU.mult)
                for ci, c in enumerate(chans):
                    last = gi == len(GROUPS) - 1 and ci == ncg - 1
                    mm_plane(psum, pr[:, ci * FLATP:(ci + 1) * FLATP],
                             OFFSETS[c][1], False, last)
        else:
            def product(c):
                dy, _ = OFFSETS[c]
                pr = prod_pool.tile([P, FLATP], F16, tag="prod", name="prod_t")
                nc.vector.tensor_tensor(
                    pr[:], gates[c],
                    rb[:, (1 + dy) * ROWW:(1 + dy) * ROWW + FLATP],
                    op=ALU.mult)
                return pr

            if split_first >= 2:
                cs = ORDERED_C[:2]
                prs = [prod_pool.tile([P, FLATP], F16, tag="prod", name="prod_t")
                       for _ in cs]
                for j in range(RP):
                    for t, c in enumerate(cs):
                        sl = slice(j * ROWW, (j + 1) * ROWW)
                        rsl = slice((1 + j) * ROWW, (2 + j) * ROWW)
                        nc.vector.tensor_tensor(prs[t][:, sl], gates[c][:, sl],
                                                rb[:, rsl], op=ALU.mult)
                for t, c in enumerate(cs):
                    mm_plane(psum, prs[t], OFFSETS[c][1], False, False)
                rest = ORDERED_C[2:]
            else:
                rest = ORDERED_C

            held = None
            body = rest[:-1] if split_last else rest
            for i, c in enumerate(body):
                is_last = (not split_last) and i == len(body) - 1
                pr = product(c)
                if merge_dx0 and c == 6:
                    held = pr          # merged into c1's plane (same dx=0)
                    continue
                if merge_dx0 and c == 1:
                    nc.vector.tensor_tensor(pr[:], pr[:], held[:], op=ALU.add)
                mm_plane(psum, pr, OFFSETS[c][1], False, is_last)

            if split_last:
                # row-split the LAST product and emit its two bank matmuls per
                # row, so the ACT copy chain (which needs banks 0,1 of all
                # planes) starts while the remaining rows still compute
                c = rest[-1]
                dy, dx = OFFSETS[c]
                pr = prod_pool.tile([P, FLATP], F16, tag="prod", name="prod_t")
                for j in range(RP):
                    sl = slice(j * ROWW, (j + 1) * ROWW)
                    nc.vector.tensor_tensor(pr[:, sl], gates[c][:, sl],
                                            rb[:, (1 + dy + j) * ROWW:
                                                (2 + dy + j) * ROWW],
                                            op=ALU.mult)
                    for q in (2 * j, 2 * j + 1):
                        off = j * ROWW + 1 + dx + (q % 2) * MMCH
                        nc.tensor.matmul(psum[:, q * BANK:q * BANK + MMCH],
                                         ident[:], pr[:, off:off + MMCH],
                                         start=False, stop=True)

        nv = rviews[nxt]
        rbn = rbufs[nxt]
        # PSUM -> next r center per row slot (ScalarE), halos via DMA.
        pv = psum[:].rearrange("p (q b) -> p q b", q=NMM)
        for j in range(RP):
            row = rbn[:, (1 + j) * ROWW + 1:(1 + j) * ROWW + 1 + W]
            nc.scalar.activation(row.rearrange("p (a b) -> p a b", a=2),
                                 pv[:, 2 * j:2 * j + 2, 0:MMCH],
                                 ACTF.Copy)
            if final:
                nc.sync.dma_start(out=out_t[:, j:j + 1, :],
                                  in_=nv[:, 1 + j:2 + j, 1:1 + W])
            elif j == 0:
                nc.sync.dma_start(out=nv[0:P - 1, RP + 1:RP + 2, :],
                                  in_=nv[1:P, 1:2, :])
        if not final:
            nc.sync.dma_start(out=nv[1:P, 0:1, :],
                              in_=nv[0:P - 1, RP:RP + 1, :])
        cur, nxt = nxt, cur

    if niter == 0:
        nc.sync.dma_start(out=out_t, in_=rviews[cur][:, 1:1 + RP, 1:1 + W])


_NC_CACHE = {}


def build_nc(prop_time: int, repeat: int = 1, split_first: int = 2,
             fuse: bool = False, merge_dx0: bool = False,
             split_last: bool = False, prod_bufs: int = 6,
             whole_repeat: int = 1):
    key = (prop_time, repeat, split_first, fuse, merge_dx0, split_last,
           prod_bufs, whole_repeat)
    if key in _NC_CACHE:
        return _NC_CACHE[key]
    nc = bacc.Bacc("TRN2", target_bir_lowering=False, debug=False)
    guid = nc.dram_tensor("guidance", [8, H, W], F16, kind="ExternalInput").ap()
    blur = nc.dram_tensor("blur", [H, W], F16, kind="ExternalInput").ap()
    ident_d = nc.dram_tensor("ident", [P, P], F16, kind="ExternalInput").ap()
    out_d = nc.dram_tensor("out", [H, W], F16, kind="ExternalOutput").ap()
    with tile.TileContext(nc) as tc, \
            nc.allow_low_precision(reason="fp16 state; tol 2e-2, measured ~1e-3"):
        for _ in range(whole_repeat):
            with ExitStack() as ctx:
                _emit(ctx, tc, guid, blur, ident_d, out_d, prop_time, repeat,
                      split_first=split_first, fuse=fuse, merge_dx0=merge_dx0,
                      split_last=split_last, prod_bufs=prod_bufs)
    nc.compile()
    _NC_CACHE[key] = nc
    return nc


def make_in_maps(guidance: np.ndarray, blur_depth: np.ndarray):
    eye = np.eye(P, dtype=np.float16)
    return [
        {
            "guidance": np.ascontiguousarray(guidance[b], dtype=np.float16),
            "blur": np.ascontiguousarray(blur_depth[b, 0], dtype=np.float16),
            "ident": eye,
        }
        for b in range(B)
    ]


def kernel(guidance, blur_depth, prop_time):
    guidance = np.asarray(guidance, dtype=np.float32)
    blur_depth = np.asarray(blur_depth, dtype=np.float32)
    pt = int(np.asarray(prop_time))
    nc = build_nc(pt)
    in_maps = make_in_maps(guidance, blur_depth)
    res = run_bass_kernel_spmd(nc, in_maps, list(range(NCORES)))
    out = np.stack([res.results[b]["out"] for b in range(B)])[:, None]
    return out.astype(np.float32)

